# revision 19
# baseline (speedup 1.0000x reference)
"""AttnDecoderLSTM Trainium2 kernel (8-core SPMD).

Structure of the computation (B=32, T=128, H=1024, V=32000):
  - The attention softmax is over a size-1 axis, so attention weights are
    exactly 1.0 and context == encoder_outputs. The Wa/Ua/Va branch never
    affects the outputs.
  - x_t = [enc, E[tok_t], enc], gates_t = x_t @ W_ih.T + h_t @ W_hh.T + b.
    The x_t part is precomputed for all t as
      g_pre[t] = E[tok_t] @ W2.T + enc @ (W1+W3).T + b_ih + b_hh
    with the emb part sharded over cores (16 steps each) and exchanged via
    one AllGather; the first 32 steps are additionally computed locally on
    every core so the recurrence can start while the AllGather is in flight.
  - The sequential recurrence (h_t -> gates -> h_{t+1}) is replicated on
    all 8 cores; the output projection (logits.T = W_out @ H2.T, 134 GMAC)
    is sharded over the vocab dim, 4000 per core, and interleaved with the
    recurrence so the TensorEngine stays busy during the elementwise chain.

Layouts:
  - fold4: a [32, 1024] batch-major tensor stored as [128, 256] SBUF tile,
    partition q*32+b holds columns 256q..256q+256. All LSTM elementwise ops
    run in fold4 so every operand sits on the same partitions.
  - gates live in 4 PSUM banks (one per gate i/f/g/o), each [128, 256] fold4,
    filled by col-tiled matmuls (tile_position=(0,32j)) with M=32; g_pre is
    accumulated into the same banks with an identity-stationary matmul.
  - h.T is maintained per step as a [128, 256] bf16 tile (col = k*32 + b)
    whose [:, 32k:32k+32] slices are the matmul stationary operands; it is
    also streamed to DRAM as [k, t, p, b] for the deferred logits matmuls.
"""

import sys
import numpy as np
import ml_dtypes

B, T, H, V = 32, 128, 1024, 32000
SOS = 1
NCORES = 8
VS = V // NCORES          # 4000 vocab per core
TB = T * B                # 4096
G = 4 * H                 # 4096 gate dims
KC = H // 128             # 8 K chunks

bf16 = ml_dtypes.bfloat16


def _fold4(x):
    """[32, 1024] -> [128, 256] fold4 layout."""
    assert x.shape == (B, H)
    return np.concatenate([x[:, q * 256:(q + 1) * 256] for q in range(4)], axis=0)


def _unfold4(x):
    """[128, 256] fold4 -> [32, 1024]."""
    return np.concatenate([x[q * 32:(q + 1) * 32] for q in range(4)], axis=1)


def _build(nsteps=T):
    import concourse.bacc as bacc
    import concourse.mybir as mybir
    import concourse.tile as tile
    from contextlib import ExitStack

    dt = mybir.dt
    nc = bacc.Bacc("TRN2", target_bir_lowering=False, debug=False,
                   num_devices=NCORES)

    TBn = nsteps * B
    n_mt = TBn // 128
    shard = (TBn % (NCORES * 128) == 0)
    TBl = TBn // NCORES if shard else TBn
    n_mtl = TBl // 128 if shard else n_mt
    N_EARLY = 1024 if shard else 0

    # ---------------- I/O ----------------
    embT_d = nc.declare_dram_parameter("embT", [H, TBl + N_EARLY], dt.bfloat16,
                                       isOutput=False)
    w2T_d = nc.declare_dram_parameter("w2T", [H, G], dt.bfloat16, isOutput=False)
    w13T_d = nc.declare_dram_parameter("w13T", [H, G], dt.bfloat16, isOutput=False)
    whhT_d = nc.declare_dram_parameter("whhT", [H, G], dt.bfloat16, isOutput=False)
    woutT_d = nc.declare_dram_parameter("woutT", [H, VS], dt.bfloat16, isOutput=False)
    bias_d = nc.declare_dram_parameter("biasrow", [1, G], dt.bfloat16, isOutput=False)
    encT_d = nc.declare_dram_parameter("encT", [H, 128], dt.bfloat16, isOutput=False)
    encf_d = nc.declare_dram_parameter("encfold", [128, 256], dt.float32,
                                       isOutput=False)
    idb_d = nc.declare_dram_parameter("identb", [128, 32], dt.bfloat16,
                                      isOutput=False)
    ones_d = nc.declare_dram_parameter("ones1", [1, 128], dt.bfloat16,
                                       isOutput=False)

    logits_d = nc.declare_dram_parameter("logits", [VS, TBn], dt.float32,
                                         isOutput=True)
    hT_o = nc.declare_dram_parameter("hT", [128, 256], dt.float32, isOutput=True)
    cT_o = nc.declare_dram_parameter("cT", [128, 256], dt.float32, isOutput=True)

    # ---------------- internal DRAM ----------------
    gpre_loc = nc.dram_tensor("gpre_loc", [TBl, G], dt.bfloat16)
    gpre_early = nc.dram_tensor("gpre_early", [max(N_EARLY, 1), G], dt.bfloat16)
    if shard:
        gpre_d = nc.dram_tensor("gpre", [TBn, G], dt.bfloat16, addr_space="Shared")
    else:
        gpre_d = gpre_loc
    h2T_d = nc.dram_tensor("h2Tbuf", [KC, nsteps, 128, 32], dt.bfloat16)

    SIG = mybir.ActivationFunctionType.Sigmoid
    TANH = mybir.ActivationFunctionType.Tanh
    COPY = mybir.ActivationFunctionType.Copy

    with tile.TileContext(nc) as tc, ExitStack() as top:
        const = top.enter_context(tc.tile_pool(name="const", bufs=1))
        idb_sb = const.tile([128, 32], dt.bfloat16)
        nc.sync.dma_start(idb_sb[:], idb_d[:])
        ones_sb = const.tile([1, 128], dt.bfloat16)
        nc.sync.dma_start(ones_sb[:], ones_d[:])
        encT_sb = [const.tile([128, 128], dt.bfloat16, name=f"encT{k}",
                              tag=f"encT{k}") for k in range(KC)]
        for k in range(KC):
            nc.sync.dma_start(encT_sb[k][:], encT_d[k * 128:(k + 1) * 128, :])
        encf_sb = const.tile([128, 256], dt.float32)
        nc.sync.dma_start(encf_sb[:], encf_d[:])
        enc_part_sb = const.tile([128, G], dt.bfloat16)

        # ---------------- P1: enc_part = enc @ (W1+W3).T + b (x4 tiled) ------
        with ExitStack() as p1:
            w13p = p1.enter_context(tc.tile_pool(name="w13", bufs=3))
            bias_sb = w13p.tile([1, G], dt.bfloat16, name="bias_sb", tag="bias",
                                bufs=1)
            nc.sync.dma_start(bias_sb[:], bias_d[:])
            ep_ps = p1.enter_context(tc.tile_pool(name="ep_ps", bufs=2,
                                                  space="PSUM"))
            for n in range(8):
                ps = ep_ps.tile([128, 512], dt.float32, name="ps_ep", tag="ps_ep")
                for k in range(KC):
                    wt = w13p.tile([128, 512], dt.bfloat16, name="w13t", tag="w13t")
                    nc.sync.dma_start(wt[:], w13T_d[k * 128:(k + 1) * 128,
                                                    n * 512:(n + 1) * 512])
                    nc.tensor.matmul(ps[:], encT_sb[k][:], wt[:],
                                     start=(k == 0), stop=False)
                nc.tensor.matmul(ps[:], ones_sb[:],
                                 bias_sb[:, n * 512:(n + 1) * 512],
                                 start=False, stop=True)
                nc.vector.tensor_copy(enc_part_sb[:, n * 512:(n + 1) * 512], ps[:])

        # ---------------- P2: g_pre = emb @ W2.T + enc_part -> DRAM ----------
        with ExitStack() as p2:
            embp = p2.enter_context(tc.tile_pool(name="embT", bufs=1))
            embT_sb = [embp.tile([128, TBl + N_EARLY], dt.bfloat16,
                                 name=f"embT{k}", tag=f"embT{k}")
                       for k in range(KC)]
            for k in range(KC):
                nc.sync.dma_start(embT_sb[k][:], embT_d[k * 128:(k + 1) * 128, :])
            w2p = p2.enter_context(tc.tile_pool(name="w2", bufs=2))
            gp_ps = p2.enter_context(tc.tile_pool(name="gp_ps", bufs=3,
                                                  space="PSUM"))
            gp_out = p2.enter_context(tc.tile_pool(name="gp_out", bufs=4))

            n_early_mt = N_EARLY // 128
            if shard:
                passes = [list(range(n_mtl, n_mtl + n_early_mt)),
                          list(range(n_mtl))]
            else:
                passes = [list(range(n_mtl))]
            for mrange in passes:
                for n in range(8):
                    w2t = [w2p.tile([128, 512], dt.bfloat16, name=f"w2t{k}",
                                    tag=f"w2t{k}") for k in range(KC)]
                    for k in range(KC):
                        nc.sync.dma_start(w2t[k][:],
                                          w2T_d[k * 128:(k + 1) * 128,
                                                n * 512:(n + 1) * 512])
                    for m in mrange:
                        ps = gp_ps.tile([128, 512], dt.float32, name="ps_gp",
                                        tag="ps_gp")
                        for k in range(KC):
                            nc.tensor.matmul(
                                ps[:], embT_sb[k][:, m * 128:(m + 1) * 128],
                                w2t[k][:], start=(k == 0), stop=(k == KC - 1))
                        ot = gp_out.tile([128, 512], dt.bfloat16, name="gp_o",
                                         tag="gp_o")
                        nc.vector.tensor_add(ot[:], ps[:],
                                             enc_part_sb[:, n * 512:(n + 1) * 512])
                        if m < n_mtl:
                            dst = gpre_loc[m * 128:(m + 1) * 128,
                                           n * 512:(n + 1) * 512]
                        else:
                            me = m - n_mtl
                            dst = gpre_early[me * 128:(me + 1) * 128,
                                             n * 512:(n + 1) * 512]
                        nc.sync.dma_start(dst, ot[:])
        if shard:
            nc.gpsimd.collective_compute(
                "AllGather", mybir.AluOpType.bypass,
                ins=[gpre_loc[:]], outs=[gpre_d[:]],
                replica_groups=[list(range(NCORES))])

        # ---------------- resident weights (scalar HWDGE queue) --------------
        wpool = top.enter_context(tc.tile_pool(name="wpool", bufs=1))
        whh_sb = [wpool.tile([128, G], dt.bfloat16, name=f"whh{k}", tag=f"whh{k}")
                  for k in range(KC)]
        for k in range(KC):
            nc.scalar.dma_start(whh_sb[k][:], whhT_d[k * 128:(k + 1) * 128, :])
        wout_sb = [wpool.tile([128, VS], dt.bfloat16, name=f"wout{k}",
                              tag=f"wout{k}") for k in range(KC)]
        for k in range(KC):
            nc.scalar.dma_start(wout_sb[k][:], woutT_d[k * 128:(k + 1) * 128, :])

        # ---------------- P3 + P4: recurrence + interleaved logits -----------
        gatep = top.enter_context(tc.tile_pool(name="gatep", bufs=1, space="PSUM"))
        trp = top.enter_context(tc.tile_pool(name="trp", bufs=1, space="PSUM"))
        lps = top.enter_context(tc.tile_pool(name="lps", bufs=2, space="PSUM"))
        gprep = top.enter_context(tc.tile_pool(name="gprep", bufs=2))
        ewp = top.enter_context(tc.tile_pool(name="ewp", bufs=2))
        cp = top.enter_context(tc.tile_pool(name="cp", bufs=2))
        hsp = top.enter_context(tc.tile_pool(name="hsp", bufs=3))
        h2tp = top.enter_context(tc.tile_pool(name="h2tp", bufs=2))
        h2t_tiles = {}            # g -> list of 8 [128, GW] tiles
        loutp = top.enter_context(tc.tile_pool(name="loutp", bufs=3))

        c_cur = encf_sb
        hstage_prev = None

        n_grp = max(TBn // 512, 1)
        GW = TBn // n_grp                   # TB cols per logits group
        VT = (VS + 127) // 128              # vocab tiles per core
        logits_units = [(g, v) for g in range(n_grp) for v in range(VT)]
        lu_idx = 0
        h2t_tiles = {}

        def hT_src(k):
            if hstage_prev is None:
                return encT_sb[k][:, 0:32]
            return hstage_prev[:, k * 32:(k + 1) * 32]

        def prefetch_group(g):
            nst = GW // B
            tiles = [h2tp.tile([128, GW], dt.bfloat16, name=f"h2t{g}_{k}",
                               tag=f"h2t{k}") for k in range(KC)]
            for k in range(KC):
                nc.scalar.dma_start(
                    tiles[k][:],
                    h2T_d[k, nst * g:nst * (g + 1), :, :].rearrange(
                        "t p b -> p t b"))
            h2t_tiles[g] = tiles

        def emit_logits_units(count):
            nonlocal lu_idx
            for i in range(count):
                if lu_idx >= len(logits_units):
                    return
                g, v = logits_units[lu_idx]
                lu_idx += 1
                tiles = h2t_tiles[g]
                M = min(128, VS - v * 128)
                ps = lps.tile([128, GW], dt.float32, name="ps_l", tag="ps_l")
                for k in range(KC):
                    nc.tensor.matmul(ps[:M, :],
                                     wout_sb[k][:, v * 128:v * 128 + M],
                                     tiles[k][:],
                                     start=(k == 0), stop=(k == KC - 1))
                ot = loutp.tile([128, GW], dt.float32, name="l_o", tag="l_o")
                if (g + v) % 2 == 0:
                    nc.scalar.activation(ot[:M, :], ps[:M, :], COPY)
                else:
                    nc.vector.tensor_copy(ot[:M, :], ps[:M, :])
                nc.sync.dma_start(
                    logits_d[v * 128:v * 128 + M, g * GW:(g + 1) * GW],
                    ot[:M, :])
                if v == VT - 1:
                    del h2t_tiles[g]

        GATE_OFF = {0: 0, 1: G // 4, 2: G // 2, 3: 3 * G // 4}  # i, f, g, o

        for t in range(nsteps):
            # logits backlog first: these fill the previous step's elementwise
            # window while the PE would otherwise idle
            nst = GW // B
            if t % nst == 0 and t > 0:
                prefetch_group(t // nst - 1)
            avail = (t // nst) * VT
            emit_logits_units(min(3, avail - lu_idx))

            gpre_t = gprep.tile([32, G], dt.bfloat16, name="gpre_t", tag="gpre_t")
            if shard and (t + 1) * 32 <= N_EARLY:
                nc.sync.dma_start(gpre_t[:], gpre_early[t * 32:(t + 1) * 32, :])
            else:
                nc.sync.dma_start(gpre_t[:], gpre_d[t * 32:(t + 1) * 32, :])

            # --- gates matmuls: 4 psum banks (i, f, g, o), fold4 each ---
            gps = [gatep.tile([128, 256], dt.float32, name=f"gps{gi}",
                              tag=f"gps{gi}") for gi in range(4)]
            crit = tc.tile_critical(sync_engine=mybir.EngineType.PE)
            crit.__enter__()
            for gi in range(4):
                for k in range(KC):
                    hsrc = hT_src(k)
                    for j in range(4):
                        nc.tensor.matmul(
                            gps[gi][j * 32:(j + 1) * 32, :],
                            hsrc,
                            whh_sb[k][:, GATE_OFF[gi] + j * 256:
                                      GATE_OFF[gi] + (j + 1) * 256],
                            start=(k == 0), stop=False, skip_group_check=True,
                            tile_position=(0, j * 32))
                for j in range(4):
                    nc.tensor.matmul(
                        gps[gi][j * 32:(j + 1) * 32, :],
                        idb_sb[0:32, :],
                        gpre_t[:, GATE_OFF[gi] + j * 256:
                               GATE_OFF[gi] + (j + 1) * 256],
                        start=False, stop=(j == 3), skip_group_check=True,
                        tile_position=(0, j * 32))
            crit.__exit__(None, None, None)

            # --- elementwise (fold4, [128, 256]) ---
            si = ewp.tile([128, 256], dt.bfloat16, name="si", tag="si")
            nc.scalar.activation(si[:], gps[0][:], SIG)
            sf = ewp.tile([128, 256], dt.bfloat16, name="sf", tag="sf")
            nc.scalar.activation(sf[:], gps[1][:], SIG)
            tg = ewp.tile([128, 256], dt.bfloat16, name="tg", tag="tg")
            nc.scalar.activation(tg[:], gps[2][:], TANH)
            so = ewp.tile([128, 256], dt.bfloat16, name="so", tag="so")
            nc.scalar.activation(so[:], gps[3][:], SIG)

            t1 = ewp.tile([128, 256], dt.float32, name="t1", tag="t1")
            nc.vector.tensor_mul(t1[:], sf[:], c_cur[:])
            t2 = ewp.tile([128, 256], dt.bfloat16, name="t2", tag="t2")
            nc.vector.tensor_mul(t2[:], si[:], tg[:])
            c_new = cp.tile([128, 256], dt.float32, name="c_new", tag="c_new")
            nc.vector.tensor_add(c_new[:], t1[:], t2[:])
            tc_ = ewp.tile([128, 256], dt.bfloat16, name="tc_", tag="tc_")
            nc.scalar.activation(tc_[:], c_new[:], TANH)
            h2 = ewp.tile([128, 256], dt.bfloat16, name="h2", tag="h2")
            nc.vector.tensor_mul(h2[:], so[:], tc_[:])
            c_cur = c_new

            # --- transpose h2 fold4 -> h.T chunks (2 waves, 2 psum tiles) ---
            hstage = hsp.tile([128, 256], dt.bfloat16, name="hstage", tag="hstage")
            for wave in range(2):
                psA = trp.tile([128, 64], dt.bfloat16, name="psA", tag="psA")
                psB = trp.tile([128, 64], dt.bfloat16, name="psB", tag="psB")
                qa, qb = 2 * wave, 2 * wave + 1
                for half in range(2):
                    nc.tensor.transpose(
                        psA[:, half * 32:(half + 1) * 32],
                        h2[qa * 32:(qa + 1) * 32, half * 128:(half + 1) * 128],
                        idb_sb[qa * 32:(qa + 1) * 32, :],
                        tile_position=(qa * 32, 0))
                    nc.tensor.transpose(
                        psB[:, half * 32:(half + 1) * 32],
                        h2[qb * 32:(qb + 1) * 32, half * 128:(half + 1) * 128],
                        idb_sb[qb * 32:(qb + 1) * 32, :],
                        tile_position=(qb * 32, 0))
                nc.vector.tensor_copy(hstage[:, wave * 128:wave * 128 + 64],
                                      psA[:])
                nc.vector.tensor_copy(hstage[:, wave * 128 + 64:wave * 128 + 128],
                                      psB[:])
            hstage_prev = hstage

            nc.sync.dma_start(
                h2T_d[:, t, :, :].rearrange("c p b -> p c b"),
                hstage[:].rearrange("p (c b) -> p c b", c=KC))

            if t == nsteps - 1:
                hf = cp.tile([128, 256], dt.float32, name="hf", tag="hf", bufs=1)
                nc.vector.tensor_copy(hf[:], h2[:])
                nc.sync.dma_start(hT_o[:], hf[:])
                nc.sync.dma_start(cT_o[:], c_new[:])

        for g in range(n_grp):
            if g not in h2t_tiles and any(u[0] == g for u in logits_units[lu_idx:]):
                prefetch_group(g)
        emit_logits_units(len(logits_units) - lu_idx)

    nc.compile()
    return nc


_PROGRAM_CACHE = {}


def _get_program(nsteps=T):
    if nsteps not in _PROGRAM_CACHE:
        _PROGRAM_CACHE[nsteps] = _build(nsteps)
    return _PROGRAM_CACHE[nsteps]


def _prep_inputs(encoder_outputs, target_tensor, E, W_ih, W_hh, b_ih, b_hh,
                 W_out, nsteps=T):
    """Host-side input prep: shifts, gather, transposes, casts, sharding."""
    enc = np.asarray(encoder_outputs, dtype=np.float32)
    tgt = np.asarray(target_tensor)
    E = np.asarray(E, dtype=np.float32)
    W_ih = np.asarray(W_ih, dtype=np.float32)
    W_hh = np.asarray(W_hh, dtype=np.float32)
    W_out = np.asarray(W_out, dtype=np.float32)
    b = (np.asarray(b_ih, dtype=np.float32) +
         np.asarray(b_hh, dtype=np.float32))

    tok = np.concatenate(
        [np.full((B, 1), SOS, dtype=np.int64), tgt[:, :-1].astype(np.int64)],
        axis=1).T[:nsteps]                  # [nsteps, B]
    emb = E[tok.reshape(-1)]                # [nsteps*B, H]
    embT = np.ascontiguousarray(emb.T).astype(bf16)

    W1 = W_ih[:, :H]
    W2 = W_ih[:, H:2 * H]
    W3 = W_ih[:, 2 * H:]
    w2T = np.ascontiguousarray(W2.T).astype(bf16)
    w13T = np.ascontiguousarray((W1 + W3).T).astype(bf16)
    whhT = np.ascontiguousarray(W_hh.T).astype(bf16)
    woutT = np.ascontiguousarray(W_out.T).astype(bf16)   # [H, V]
    encT = np.tile(np.ascontiguousarray(enc.T).astype(bf16), (1, 4))
    encf = _fold4(enc).astype(np.float32)
    identb = np.tile(np.eye(32, dtype=bf16), (4, 1))
    ones1 = np.ones((1, 128), dtype=bf16)
    biasrow = b[None, :].astype(bf16)

    base = dict(w2T=w2T, w13T=w13T, whhT=whhT, biasrow=biasrow,
                encT=encT, encfold=encf, identb=identb, ones1=ones1)
    TBn = nsteps * B
    shard = (TBn % (NCORES * 128) == 0)
    TBl = TBn // NCORES if shard else TBn
    in_maps = []
    for c in range(NCORES):
        m = dict(base)
        m["woutT"] = np.ascontiguousarray(woutT[:, c * VS:(c + 1) * VS])
        if shard:
            m["embT"] = np.ascontiguousarray(np.concatenate(
                [embT[:, c * TBl:(c + 1) * TBl], embT[:, :1024]], axis=1))
        else:
            m["embT"] = embT
        in_maps.append(m)
    return in_maps


def _assemble(results, b_out, nsteps=T):
    """Gather per-core outputs into the reference's output pytree."""
    logits = np.concatenate([results[c]["logits"] for c in range(NCORES)],
                            axis=0)                      # [V, nsteps*B]
    dec = logits.reshape(V, nsteps, B).transpose(2, 1, 0)  # [B, nsteps, V]
    b_out = np.asarray(b_out, dtype=np.float32)
    if np.any(b_out):
        dec = dec + b_out[None, None, :]
    hT = _unfold4(results[0]["hT"])[None]                # [1, B, H]
    cT = _unfold4(results[0]["cT"])[None]
    attn = np.ones((B, nsteps, 1), dtype=np.float32)
    return np.ascontiguousarray(dec, dtype=np.float32), (
        hT.astype(np.float32), cT.astype(np.float32)), attn


def kernel(encoder_outputs, target_tensor, E, Wa, b_wa, Ua, b_ua, Va, b_va,
           W_ih, W_hh, b_ih, b_hh, W_out, b_out,
           _nsteps=T, _trace=False, _return_info=False):
    sys.path.insert(0, "/opt/trn_rl_repo")
    from concourse.bass_utils import run_bass_kernel_spmd

    nc = _get_program(_nsteps)
    in_maps = _prep_inputs(encoder_outputs, target_tensor, E, W_ih, W_hh,
                           b_ih, b_hh, W_out, nsteps=_nsteps)
    res = run_bass_kernel_spmd(nc, in_maps, core_ids=list(range(NCORES)),
                               trace=_trace)
    out = _assemble(res.results, b_out, nsteps=_nsteps)
    if _return_info:
        return out, {"exec_time_ns": res.exec_time_ns}
    return out


# revision 20
# speedup vs baseline: 1.4215x; 1.4215x over previous
"""AttnDecoderLSTM Trainium2 kernel (8-core SPMD).

Structure of the computation (B=32, T=128, H=1024, V=32000):
  - The attention softmax is over a size-1 axis, so attention weights are
    exactly 1.0 and context == encoder_outputs. The Wa/Ua/Va branch never
    affects the outputs.
  - x_t = [enc, E[tok_t], enc], gates_t = x_t @ W_ih.T + h_t @ W_hh.T + b.
    The x_t part is precomputed for all t as
      g_pre[t] = E[tok_t] @ W2.T + enc @ (W1+W3).T + b_ih + b_hh
    with the emb part sharded over cores (16 steps each) and exchanged via
    one AllGather; the first 32 steps are additionally computed locally on
    every core so the recurrence can start while the AllGather is in flight.
  - The sequential recurrence (h_t -> gates -> h_{t+1}) is replicated on
    all 8 cores; the output projection (logits.T = W_out @ H2.T, 134 GMAC)
    is sharded over the vocab dim, 4000 per core, and interleaved with the
    recurrence so the TensorEngine stays busy during the elementwise chain.

Layouts:
  - fold4: a [32, 1024] batch-major tensor stored as [128, 256] SBUF tile,
    partition q*32+b holds columns 256q..256q+256. All LSTM elementwise ops
    run in fold4 so every operand sits on the same partitions.
  - gates live in 4 PSUM banks (one per gate i/f/g/o), each [128, 256] fold4,
    filled by col-tiled matmuls (tile_position=(0,32j)) with M=32; g_pre is
    accumulated into the same banks with an identity-stationary matmul.
  - h.T is maintained per step as a [128, 256] bf16 tile (col = k*32 + b)
    whose [:, 32k:32k+32] slices are the matmul stationary operands; it is
    also streamed to DRAM as [k, t, p, b] for the deferred logits matmuls.
"""

import sys
import numpy as np
import ml_dtypes

B, T, H, V = 32, 128, 1024, 32000
SOS = 1
NCORES = 8
VS = V // NCORES          # 4000 vocab per core
TB = T * B                # 4096
G = 4 * H                 # 4096 gate dims
KC = H // 128             # 8 K chunks

bf16 = ml_dtypes.bfloat16


def _fold4(x):
    """[32, 1024] -> [128, 256] fold4 layout."""
    assert x.shape == (B, H)
    return np.concatenate([x[:, q * 256:(q + 1) * 256] for q in range(4)], axis=0)


def _unfold4(x):
    """[128, 256] fold4 -> [32, 1024]."""
    return np.concatenate([x[q * 32:(q + 1) * 32] for q in range(4)], axis=1)


def _build(nsteps=T):
    import concourse.bacc as bacc
    import concourse.mybir as mybir
    import concourse.tile as tile
    from contextlib import ExitStack

    dt = mybir.dt
    nc = bacc.Bacc("TRN2", target_bir_lowering=False, debug=False,
                   num_devices=NCORES)

    TBn = nsteps * B
    n_mt = TBn // 128
    shard = (TBn % (NCORES * 128) == 0)
    TBl = TBn // NCORES if shard else TBn
    n_mtl = TBl // 128 if shard else n_mt
    N_EARLY = 1024 if shard else 0

    # ---------------- I/O ----------------
    embT_d = nc.declare_dram_parameter("embT", [H, TBl + N_EARLY], dt.bfloat16,
                                       isOutput=False)
    w2T_d = nc.declare_dram_parameter("w2T", [H, G], dt.bfloat16, isOutput=False)
    w13T_d = nc.declare_dram_parameter("w13T", [H, G], dt.bfloat16, isOutput=False)
    whhT_d = nc.declare_dram_parameter("whhT", [H, G], dt.bfloat16, isOutput=False)
    woutT_d = nc.declare_dram_parameter("woutT", [H, VS], dt.bfloat16, isOutput=False)
    bias_d = nc.declare_dram_parameter("biasrow", [1, G], dt.bfloat16, isOutput=False)
    encT_d = nc.declare_dram_parameter("encT", [H, 128], dt.bfloat16, isOutput=False)
    encf_d = nc.declare_dram_parameter("encfold", [128, 256], dt.float32,
                                       isOutput=False)
    idb_d = nc.declare_dram_parameter("identb", [128, 32], dt.bfloat16,
                                      isOutput=False)
    ones_d = nc.declare_dram_parameter("ones1", [1, 128], dt.bfloat16,
                                       isOutput=False)

    logits_d = nc.declare_dram_parameter("logits", [VS, TBn], dt.float32,
                                         isOutput=True)
    hT_o = nc.declare_dram_parameter("hT", [128, 256], dt.float32, isOutput=True)
    cT_o = nc.declare_dram_parameter("cT", [128, 256], dt.float32, isOutput=True)

    # ---------------- internal DRAM ----------------
    gpre_loc = nc.dram_tensor("gpre_loc", [TBl, G], dt.bfloat16)
    gpre_early = nc.dram_tensor("gpre_early", [max(N_EARLY, 1), G], dt.bfloat16)
    if shard:
        gpre_d = nc.dram_tensor("gpre", [TBn, G], dt.bfloat16, addr_space="Shared")
    else:
        gpre_d = gpre_loc
    h2T_d = nc.dram_tensor("h2Tbuf", [KC, nsteps, 128, 32], dt.bfloat16)

    SIG = mybir.ActivationFunctionType.Sigmoid
    TANH = mybir.ActivationFunctionType.Tanh
    COPY = mybir.ActivationFunctionType.Copy

    with tile.TileContext(nc) as tc, ExitStack() as top:
        const = top.enter_context(tc.tile_pool(name="const", bufs=1))
        idb_sb = const.tile([128, 32], dt.bfloat16)
        nc.sync.dma_start(idb_sb[:], idb_d[:])
        ones_sb = const.tile([1, 128], dt.bfloat16)
        nc.sync.dma_start(ones_sb[:], ones_d[:])
        encT_sb = [const.tile([128, 128], dt.bfloat16, name=f"encT{k}",
                              tag=f"encT{k}") for k in range(KC)]
        for k in range(KC):
            nc.sync.dma_start(encT_sb[k][:], encT_d[k * 128:(k + 1) * 128, :])
        encf_sb = const.tile([128, 256], dt.float32)
        nc.sync.dma_start(encf_sb[:], encf_d[:])
        enc_part_sb = const.tile([128, G], dt.bfloat16)

        # ---------------- P1: enc_part = enc @ (W1+W3).T + b (x4 tiled) ------
        with ExitStack() as p1:
            w13p = p1.enter_context(tc.tile_pool(name="w13", bufs=3))
            bias_sb = w13p.tile([1, G], dt.bfloat16, name="bias_sb", tag="bias",
                                bufs=1)
            nc.sync.dma_start(bias_sb[:], bias_d[:])
            ep_ps = p1.enter_context(tc.tile_pool(name="ep_ps", bufs=2,
                                                  space="PSUM"))
            for n in range(8):
                ps = ep_ps.tile([128, 512], dt.float32, name="ps_ep", tag="ps_ep")
                for k in range(KC):
                    wt = w13p.tile([128, 512], dt.bfloat16, name="w13t", tag="w13t")
                    nc.sync.dma_start(wt[:], w13T_d[k * 128:(k + 1) * 128,
                                                    n * 512:(n + 1) * 512])
                    nc.tensor.matmul(ps[:], encT_sb[k][:], wt[:],
                                     start=(k == 0), stop=False)
                nc.tensor.matmul(ps[:], ones_sb[:],
                                 bias_sb[:, n * 512:(n + 1) * 512],
                                 start=False, stop=True)
                nc.vector.tensor_copy(enc_part_sb[:, n * 512:(n + 1) * 512], ps[:])

        # ---------------- P2: g_pre = emb @ W2.T + enc_part -> DRAM ----------
        with ExitStack() as p2:
            embp = p2.enter_context(tc.tile_pool(name="embT", bufs=1))
            embT_sb = [embp.tile([128, TBl + N_EARLY], dt.bfloat16,
                                 name=f"embT{k}", tag=f"embT{k}")
                       for k in range(KC)]
            for k in range(KC):
                nc.sync.dma_start(embT_sb[k][:], embT_d[k * 128:(k + 1) * 128, :])
            w2p = p2.enter_context(tc.tile_pool(name="w2", bufs=2))
            gp_ps = p2.enter_context(tc.tile_pool(name="gp_ps", bufs=3,
                                                  space="PSUM"))
            gp_out = p2.enter_context(tc.tile_pool(name="gp_out", bufs=4))

            n_early_mt = N_EARLY // 128
            if shard:
                passes = [list(range(n_mtl, n_mtl + n_early_mt)),
                          list(range(n_mtl))]
            else:
                passes = [list(range(n_mtl))]
            for mrange in passes:
                for n in range(8):
                    w2t = [w2p.tile([128, 512], dt.bfloat16, name=f"w2t{k}",
                                    tag=f"w2t{k}") for k in range(KC)]
                    for k in range(KC):
                        nc.sync.dma_start(w2t[k][:],
                                          w2T_d[k * 128:(k + 1) * 128,
                                                n * 512:(n + 1) * 512])
                    for m in mrange:
                        ps = gp_ps.tile([128, 512], dt.float32, name="ps_gp",
                                        tag="ps_gp")
                        for k in range(KC):
                            nc.tensor.matmul(
                                ps[:], embT_sb[k][:, m * 128:(m + 1) * 128],
                                w2t[k][:], start=(k == 0), stop=(k == KC - 1))
                        ot = gp_out.tile([128, 512], dt.bfloat16, name="gp_o",
                                         tag="gp_o")
                        nc.vector.tensor_add(ot[:], ps[:],
                                             enc_part_sb[:, n * 512:(n + 1) * 512])
                        if m < n_mtl:
                            dst = gpre_loc[m * 128:(m + 1) * 128,
                                           n * 512:(n + 1) * 512]
                        else:
                            me = m - n_mtl
                            dst = gpre_early[me * 128:(me + 1) * 128,
                                             n * 512:(n + 1) * 512]
                        nc.sync.dma_start(dst, ot[:])
        if shard:
            nc.gpsimd.collective_compute(
                "AllGather", mybir.AluOpType.bypass,
                ins=[gpre_loc[:]], outs=[gpre_d[:]],
                replica_groups=[list(range(NCORES))])

        # ---------------- resident weights (scalar HWDGE queue) --------------
        wpool = top.enter_context(tc.tile_pool(name="wpool", bufs=1))
        whh_sb = [wpool.tile([128, G], dt.bfloat16, name=f"whh{k}", tag=f"whh{k}")
                  for k in range(KC)]
        for k in range(KC):
            nc.scalar.dma_start(whh_sb[k][:], whhT_d[k * 128:(k + 1) * 128, :])
        wout_sb = [wpool.tile([128, VS], dt.bfloat16, name=f"wout{k}",
                              tag=f"wout{k}") for k in range(KC)]
        for k in range(KC):
            nc.scalar.dma_start(wout_sb[k][:], woutT_d[k * 128:(k + 1) * 128, :])

        # ---------------- P3 + P4: recurrence + interleaved logits -----------
        gatep = top.enter_context(tc.tile_pool(name="gatep", bufs=1, space="PSUM"))
        trp = top.enter_context(tc.tile_pool(name="trp", bufs=1, space="PSUM"))
        lps = top.enter_context(tc.tile_pool(name="lps", bufs=2, space="PSUM"))
        gprep = top.enter_context(tc.tile_pool(name="gprep", bufs=2))
        ewp = top.enter_context(tc.tile_pool(name="ewp", bufs=2))
        cp = top.enter_context(tc.tile_pool(name="cp", bufs=2))
        hsp = top.enter_context(tc.tile_pool(name="hsp", bufs=3))
        h2tp = top.enter_context(tc.tile_pool(name="h2tp", bufs=2))
        h2t_tiles = {}            # g -> list of 8 [128, GW] tiles
        loutp = top.enter_context(tc.tile_pool(name="loutp", bufs=3))

        c_cur = encf_sb
        hstage_prev = None

        n_grp = max(TBn // 512, 1)
        GW = TBn // n_grp                   # TB cols per logits group
        VT = (VS + 127) // 128              # vocab tiles per core
        logits_units = [(g, v) for g in range(n_grp) for v in range(VT)]
        lu_idx = 0
        h2t_tiles = {}

        def hT_src(k):
            if hstage_prev is None:
                return encT_sb[k][:, 0:32]
            return hstage_prev[:, k * 32:(k + 1) * 32]

        def prefetch_group(g):
            nst = GW // B
            tiles = [h2tp.tile([128, GW], dt.bfloat16, name=f"h2t{g}_{k}",
                               tag=f"h2t{k}") for k in range(KC)]
            for k in range(KC):
                nc.scalar.dma_start(
                    tiles[k][:],
                    h2T_d[k, nst * g:nst * (g + 1), :, :].rearrange(
                        "t p b -> p t b"))
            h2t_tiles[g] = tiles

        def emit_logits_units(count):
            nonlocal lu_idx
            for i in range(count):
                if lu_idx >= len(logits_units):
                    return
                g, v = logits_units[lu_idx]
                lu_idx += 1
                tiles = h2t_tiles[g]
                M = min(128, VS - v * 128)
                ps = lps.tile([128, GW], dt.float32, name="ps_l", tag="ps_l")
                for k in range(KC):
                    nc.tensor.matmul(ps[:M, :],
                                     wout_sb[k][:, v * 128:v * 128 + M],
                                     tiles[k][:],
                                     start=(k == 0), stop=(k == KC - 1))
                ot = loutp.tile([128, GW], dt.float32, name="l_o", tag="l_o")
                if (g + v) % 2 == 0:
                    nc.scalar.activation(ot[:M, :], ps[:M, :], COPY)
                else:
                    nc.vector.tensor_copy(ot[:M, :], ps[:M, :])
                nc.sync.dma_start(
                    logits_d[v * 128:v * 128 + M, g * GW:(g + 1) * GW],
                    ot[:M, :])
                if v == VT - 1:
                    del h2t_tiles[g]

        GATE_OFF = {0: 0, 1: G // 4, 2: G // 2, 3: 3 * G // 4}  # i, f, g, o

        for t in range(nsteps):
            # logits backlog first: these fill the previous step's elementwise
            # window while the PE would otherwise idle
            nst = GW // B
            if t % nst == 0 and t > 0:
                prefetch_group(t // nst - 1)
            avail = (t // nst) * VT
            emit_logits_units(min(3, avail - lu_idx))

            gpre_t = gprep.tile([32, G], dt.bfloat16, name="gpre_t", tag="gpre_t")
            if shard and (t + 1) * 32 <= N_EARLY:
                nc.sync.dma_start(gpre_t[:], gpre_early[t * 32:(t + 1) * 32, :])
            else:
                nc.sync.dma_start(gpre_t[:], gpre_d[t * 32:(t + 1) * 32, :])

            # --- gates matmuls: 4 psum banks (i, f, g, o), fold4 each ---
            gps = [gatep.tile([128, 256], dt.float32, name=f"gps{gi}",
                              tag=f"gps{gi}") for gi in range(4)]
            for gi in range(4):
                for k in range(KC):
                    hsrc = hT_src(k)
                    for j in range(4):
                        nc.tensor.matmul(
                            gps[gi][j * 32:(j + 1) * 32, :],
                            hsrc,
                            whh_sb[k][:, GATE_OFF[gi] + j * 256:
                                      GATE_OFF[gi] + (j + 1) * 256],
                            start=(k == 0), stop=False, skip_group_check=True,
                            tile_position=(0, j * 32))
                for j in range(4):
                    nc.tensor.matmul(
                        gps[gi][j * 32:(j + 1) * 32, :],
                        idb_sb[0:32, :],
                        gpre_t[:, GATE_OFF[gi] + j * 256:
                               GATE_OFF[gi] + (j + 1) * 256],
                        start=False, stop=(j == 3), skip_group_check=True,
                        tile_position=(0, j * 32))

            # --- elementwise (fold4, [128, 256]) ---
            si = ewp.tile([128, 256], dt.bfloat16, name="si", tag="si")
            nc.scalar.activation(si[:], gps[0][:], SIG)
            sf = ewp.tile([128, 256], dt.bfloat16, name="sf", tag="sf")
            nc.scalar.activation(sf[:], gps[1][:], SIG)
            tg = ewp.tile([128, 256], dt.bfloat16, name="tg", tag="tg")
            nc.scalar.activation(tg[:], gps[2][:], TANH)
            so = ewp.tile([128, 256], dt.bfloat16, name="so", tag="so")
            nc.scalar.activation(so[:], gps[3][:], SIG)

            t1 = ewp.tile([128, 256], dt.float32, name="t1", tag="t1")
            nc.vector.tensor_mul(t1[:], sf[:], c_cur[:])
            t2 = ewp.tile([128, 256], dt.bfloat16, name="t2", tag="t2")
            nc.vector.tensor_mul(t2[:], si[:], tg[:])
            c_new = cp.tile([128, 256], dt.float32, name="c_new", tag="c_new")
            nc.vector.tensor_add(c_new[:], t1[:], t2[:])
            tc_ = ewp.tile([128, 256], dt.bfloat16, name="tc_", tag="tc_")
            nc.scalar.activation(tc_[:], c_new[:], TANH)
            h2 = ewp.tile([128, 256], dt.bfloat16, name="h2", tag="h2")
            nc.vector.tensor_mul(h2[:], so[:], tc_[:])
            c_cur = c_new

            # --- transpose h2 fold4 -> h.T chunks (2 waves, 2 psum tiles) ---
            hstage = hsp.tile([128, 256], dt.bfloat16, name="hstage", tag="hstage")
            for wave in range(2):
                psA = trp.tile([128, 64], dt.bfloat16, name="psA", tag="psA")
                psB = trp.tile([128, 64], dt.bfloat16, name="psB", tag="psB")
                qa, qb = 2 * wave, 2 * wave + 1
                for half in range(2):
                    nc.tensor.transpose(
                        psA[:, half * 32:(half + 1) * 32],
                        h2[qa * 32:(qa + 1) * 32, half * 128:(half + 1) * 128],
                        idb_sb[qa * 32:(qa + 1) * 32, :],
                        tile_position=(qa * 32, 0))
                    nc.tensor.transpose(
                        psB[:, half * 32:(half + 1) * 32],
                        h2[qb * 32:(qb + 1) * 32, half * 128:(half + 1) * 128],
                        idb_sb[qb * 32:(qb + 1) * 32, :],
                        tile_position=(qb * 32, 0))
                nc.vector.tensor_copy(hstage[:, wave * 128:wave * 128 + 64],
                                      psA[:])
                nc.vector.tensor_copy(hstage[:, wave * 128 + 64:wave * 128 + 128],
                                      psB[:])
            hstage_prev = hstage

            nc.sync.dma_start(
                h2T_d[:, t, :, :].rearrange("c p b -> p c b"),
                hstage[:].rearrange("p (c b) -> p c b", c=KC))

            if t == nsteps - 1:
                hf = cp.tile([128, 256], dt.float32, name="hf", tag="hf", bufs=1)
                nc.vector.tensor_copy(hf[:], h2[:])
                nc.sync.dma_start(hT_o[:], hf[:])
                nc.sync.dma_start(cT_o[:], c_new[:])

        for g in range(n_grp):
            if g not in h2t_tiles and any(u[0] == g for u in logits_units[lu_idx:]):
                prefetch_group(g)
        emit_logits_units(len(logits_units) - lu_idx)

    nc.compile()
    return nc


_PROGRAM_CACHE = {}


def _get_program(nsteps=T):
    if nsteps not in _PROGRAM_CACHE:
        _PROGRAM_CACHE[nsteps] = _build(nsteps)
    return _PROGRAM_CACHE[nsteps]


def _prep_inputs(encoder_outputs, target_tensor, E, W_ih, W_hh, b_ih, b_hh,
                 W_out, nsteps=T):
    """Host-side input prep: shifts, gather, transposes, casts, sharding."""
    enc = np.asarray(encoder_outputs, dtype=np.float32)
    tgt = np.asarray(target_tensor)
    E = np.asarray(E, dtype=np.float32)
    W_ih = np.asarray(W_ih, dtype=np.float32)
    W_hh = np.asarray(W_hh, dtype=np.float32)
    W_out = np.asarray(W_out, dtype=np.float32)
    b = (np.asarray(b_ih, dtype=np.float32) +
         np.asarray(b_hh, dtype=np.float32))

    tok = np.concatenate(
        [np.full((B, 1), SOS, dtype=np.int64), tgt[:, :-1].astype(np.int64)],
        axis=1).T[:nsteps]                  # [nsteps, B]
    emb = E[tok.reshape(-1)]                # [nsteps*B, H]
    embT = np.ascontiguousarray(emb.T).astype(bf16)

    W1 = W_ih[:, :H]
    W2 = W_ih[:, H:2 * H]
    W3 = W_ih[:, 2 * H:]
    w2T = np.ascontiguousarray(W2.T).astype(bf16)
    w13T = np.ascontiguousarray((W1 + W3).T).astype(bf16)
    whhT = np.ascontiguousarray(W_hh.T).astype(bf16)
    woutT = np.ascontiguousarray(W_out.T).astype(bf16)   # [H, V]
    encT = np.tile(np.ascontiguousarray(enc.T).astype(bf16), (1, 4))
    encf = _fold4(enc).astype(np.float32)
    identb = np.tile(np.eye(32, dtype=bf16), (4, 1))
    ones1 = np.ones((1, 128), dtype=bf16)
    biasrow = b[None, :].astype(bf16)

    base = dict(w2T=w2T, w13T=w13T, whhT=whhT, biasrow=biasrow,
                encT=encT, encfold=encf, identb=identb, ones1=ones1)
    TBn = nsteps * B
    shard = (TBn % (NCORES * 128) == 0)
    TBl = TBn // NCORES if shard else TBn
    in_maps = []
    for c in range(NCORES):
        m = dict(base)
        m["woutT"] = np.ascontiguousarray(woutT[:, c * VS:(c + 1) * VS])
        if shard:
            m["embT"] = np.ascontiguousarray(np.concatenate(
                [embT[:, c * TBl:(c + 1) * TBl], embT[:, :1024]], axis=1))
        else:
            m["embT"] = embT
        in_maps.append(m)
    return in_maps


def _assemble(results, b_out, nsteps=T):
    """Gather per-core outputs into the reference's output pytree."""
    logits = np.concatenate([results[c]["logits"] for c in range(NCORES)],
                            axis=0)                      # [V, nsteps*B]
    dec = logits.reshape(V, nsteps, B).transpose(2, 1, 0)  # [B, nsteps, V]
    b_out = np.asarray(b_out, dtype=np.float32)
    if np.any(b_out):
        dec = dec + b_out[None, None, :]
    hT = _unfold4(results[0]["hT"])[None]                # [1, B, H]
    cT = _unfold4(results[0]["cT"])[None]
    attn = np.ones((B, nsteps, 1), dtype=np.float32)
    return np.ascontiguousarray(dec, dtype=np.float32), (
        hT.astype(np.float32), cT.astype(np.float32)), attn


def kernel(encoder_outputs, target_tensor, E, Wa, b_wa, Ua, b_ua, Va, b_va,
           W_ih, W_hh, b_ih, b_hh, W_out, b_out,
           _nsteps=T, _trace=False, _return_info=False):
    sys.path.insert(0, "/opt/trn_rl_repo")
    from concourse.bass_utils import run_bass_kernel_spmd

    nc = _get_program(_nsteps)
    in_maps = _prep_inputs(encoder_outputs, target_tensor, E, W_ih, W_hh,
                           b_ih, b_hh, W_out, nsteps=_nsteps)
    res = run_bass_kernel_spmd(nc, in_maps, core_ids=list(range(NCORES)),
                               trace=_trace)
    out = _assemble(res.results, b_out, nsteps=_nsteps)
    if _return_info:
        return out, {"exec_time_ns": res.exec_time_ns}
    return out
